# revision 1
# baseline (speedup 1.0000x reference)
"""Trainium2 Bass kernel for nn_CPWGenerator (B=16384, D=128, P=10, F=1024).

Data-parallel over batch across 8 NeuronCores (2048 rows/core). Per core:
  - feature-major 3-layer MLPs (control-point head + weight head)
  - softmax denominator cancels: out = (basis @ (e*cpm)) / (basis @ e)
    with e = exp(logits) raw (scale-invariant; the reference's +1e-8 eps
    term shifts the result by <1.1e-7 of scale here — measured — because
    den >= 0.07)
  - final basis matmuls produce batch-major [128, F] output tiles directly
  - division: reciprocal_approx_fast (DVE) + multiply (DVE/GPSIMD split)
Matmuls run as float32r (fp32 storage, 11-bit-mantissa operand rounding,
exact fp32 accumulation) at full PE rate.
"""
import sys
if "/opt/trn_rl_repo" not in sys.path:
    sys.path.insert(0, "/opt/trn_rl_repo")

from contextlib import ExitStack

import numpy as np

import concourse.bacc as bacc
import concourse.mybir as mybir
import concourse.tile as tile
from concourse.bass_utils import run_bass_kernel_spmd

F32 = mybir.dt.float32
F32R = mybir.dt.float32r
AF = mybir.ActivationFunctionType

# problem shapes (hardcoded per contest contract)
B, D, P, F = 16384, 128, 10, 1024
NCORES = 8
BC = B // NCORES          # rows per core = 2048
BLOCKS = [512, 512, 512, 512]   # batch blocks (sum = BC)
EPS = 1e-8

# (block, j) pairs whose final multiply runs on GPSIMD (ACT copies the
# numerator out of PSUM first); the rest multiply on DVE straight from PSUM.
GP_MUL = {(0, 1), (0, 3), (1, 1), (1, 3),
          (2, 1), (2, 3), (3, 1), (3, 2), (3, 3)}
# (block, j) pairs whose output DMA issues from GPSIMD (SWDGE) instead of
# the SP HWDGE queue, to spread DMA issue across queues.
GP_DMA = set()

# f32r const blob column offsets
_C_W1T = 0            # [128 x 128]
_C_W2T = 128          # [128 x 256]
_C_W3T = 384          # [128 x 40]  (W3Ta | W3Tb, 20 cols each)
_C_WW1T = 424         # [128 x 64]
_C_WW2T = 488         # [64  x 128]
_C_WW3T = 616         # [128 x 10]
_C_P20 = 626          # [20  x 10]  pairing matrix (0.5 per pair)
C_R = 636

# fp32 const blob columns
_C_ID = 0             # [128 x 128] identity
_C_B1 = 128
_C_B2A = 129
_C_B2B = 130
_C_B3 = 131
_C_WB1 = 132
_C_WB2 = 133
_C_WB3 = 134
C_F = 135


def round_f32r(x: np.ndarray) -> np.ndarray:
    """fp32 -> fp32r rounding (keep 11 explicit mantissa bits, RNE).
    Matches TRN2 hardware exactly (validated on device)."""
    u = np.ascontiguousarray(x, dtype=np.float32).view(np.uint32)
    keep = np.uint32(0xFFFFF000)
    half = np.uint32(0x800)
    lsb = (u >> np.uint32(12)) & np.uint32(1)
    r = (u + half - np.uint32(1) + lsb) & keep
    return r.view(np.float32)


def basis_matrix() -> np.ndarray:
    """Replicates reference._basis_matrix in float32."""
    t = np.linspace(0.0, 1.0, F, dtype=np.float32)
    centers = (np.arange(P, dtype=np.float32) / np.float32(P - 1))
    sigma = np.float32(1.0 / P)
    z = (t[:, None] - centers[None, :]).astype(np.float32)
    basis = np.exp(-(z * z) / (np.float32(2.0) * sigma * sigma),
                   dtype=np.float32)
    return basis / (basis.sum(axis=1, keepdims=True) + np.float32(EPS))


def build_program():
    nc = bacc.Bacc()
    x_in = nc.declare_dram_parameter("x", [BC, D], F32, isOutput=False)
    wr_in = nc.declare_dram_parameter("wr", [128, C_R], F32R, isOutput=False)
    bt_in = nc.declare_dram_parameter("bt", [P, F], F32R, isOutput=False)
    wf_in = nc.declare_dram_parameter("wf", [128, C_F], F32, isOutput=False)
    out = nc.declare_dram_parameter("out", [BC, F], F32, isOutput=True)

    with tile.TileContext(nc) as tc, ExitStack() as ctx:
        cpool = ctx.enter_context(tc.tile_pool(name="const", bufs=1))
        wpool = ctx.enter_context(tc.tile_pool(name="work", bufs=2))
        npool = ctx.enter_context(tc.tile_pool(name="numcp", bufs=2))
        rpool = ctx.enter_context(tc.tile_pool(name="recip", bufs=2))
        opool = ctx.enter_context(tc.tile_pool(name="outp", bufs=4))
        ppool = ctx.enter_context(tc.tile_pool(name="psum", bufs=4, space="PSUM"))
        qpool = ctx.enter_context(tc.tile_pool(name="psumo", bufs=2, space="PSUM"))

        wr = cpool.tile([128, C_R], F32R)
        bt = cpool.tile([P, F], F32R)
        wf = cpool.tile([128, C_F], F32)
        xall = cpool.tile([128, BC], F32)

        def x_dma(xoff, nb_):
            nc.gpsimd.dma_start(
                xall[:, xoff:xoff + nb_].rearrange(
                    "p (c d) -> p c d", c=nb_ // 128),
                x_in[xoff:xoff + nb_, :].rearrange(
                    "(c p) d -> p c d", p=128),
            )

        # in-DMA order tuned for pipeline fill: identity+biases first (gates
        # the first transpose), then x block 0, then weights, then the rest
        nc.gpsimd.dma_start(wf[:], wf_in[:])
        x_dma(0, BLOCKS[0])
        nc.gpsimd.dma_start(wr[:], wr_in[:])
        nc.gpsimd.dma_start(bt[:], bt_in[:])
        xoff = BLOCKS[0]
        for nb_ in BLOCKS[1:]:
            x_dma(xoff, nb_)
            xoff += nb_

        ident = wf[:, _C_ID:_C_ID + 128]

        def mm(out_ap, lhsT, rhs, start=True, stop=True):
            nc.tensor.matmul(out_ap, lhsT, rhs, start=start, stop=stop)

        x0 = 0
        for blk, NB in enumerate(BLOCKS):

            # --- transpose x block: [128b,128d] chunks -> xT [128d, NB b]
            xtp = ppool.tile([128, NB], F32, tag="ps")
            for c in range(NB // 128):
                nc.tensor.matmul(
                    xtp[:, 128 * c:128 * (c + 1)],
                    xall[:, x0 + 128 * c:x0 + 128 * (c + 1)],
                    ident,
                    is_transpose=True,
                    start=(c % 4 == 0),
                    stop=(c % 4 == 3),
                )
            xt = wpool.tile([128, NB], F32R)
            nc.scalar.activation(xt[:], xtp[:], AF.Copy)

            # --- cp MLP (feature-major)
            h1p = ppool.tile([128, NB], F32, tag="ps")
            for n in range(NB // 512):
                mm(h1p[:, 512 * n:512 * (n + 1)],
                   wr[:, _C_W1T:_C_W1T + 128],
                   xt[:, 512 * n:512 * (n + 1)])
            h1 = wpool.tile([128, NB], F32R)
            nc.scalar.activation(h1[:], h1p[:], AF.Relu,
                                 bias=wf[:, _C_B1:_C_B1 + 1])

            h2pa = ppool.tile([128, NB], F32, tag="ps")
            for n in range(NB // 512):
                mm(h2pa[:, 512 * n:512 * (n + 1)],
                   wr[:, _C_W2T:_C_W2T + 128],
                   h1[:, 512 * n:512 * (n + 1)])
            h2a = wpool.tile([128, NB], F32R)
            nc.scalar.activation(h2a[:], h2pa[:], AF.Relu,
                                 bias=wf[:, _C_B2A:_C_B2A + 1])

            h2pb = ppool.tile([128, NB], F32, tag="ps")
            for n in range(NB // 512):
                mm(h2pb[:, 512 * n:512 * (n + 1)],
                   wr[:, _C_W2T + 128:_C_W2T + 256],
                   h1[:, 512 * n:512 * (n + 1)])
            h2b = wpool.tile([128, NB], F32R)
            nc.scalar.activation(h2b[:], h2pb[:], AF.Relu,
                                 bias=wf[:, _C_B2B:_C_B2B + 1])

            cpp = ppool.tile([20, NB], F32, tag="ps")
            for n in range(NB // 512):
                sl = slice(512 * n, 512 * (n + 1))
                mm(cpp[:, sl], wr[:, _C_W3T:_C_W3T + 20], h2a[:, sl],
                   stop=False)
                mm(cpp[:, sl], wr[:, _C_W3T + 20:_C_W3T + 40], h2b[:, sl],
                   start=False, stop=True)
            cp = wpool.tile([20, NB], F32R)
            nc.scalar.activation(cp[:], cpp[:], AF.Tanh,
                                 bias=wf[0:20, _C_B3:_C_B3 + 1])

            # --- w MLP
            g1p = ppool.tile([64, NB], F32, tag="ps")
            for n in range(NB // 512):
                mm(g1p[:, 512 * n:512 * (n + 1)],
                   wr[:, _C_WW1T:_C_WW1T + 64],
                   xt[:, 512 * n:512 * (n + 1)])
            g1 = wpool.tile([64, NB], F32R)
            nc.scalar.activation(g1[:], g1p[:], AF.Relu,
                                 bias=wf[0:64, _C_WB1:_C_WB1 + 1])

            g2p = ppool.tile([128, NB], F32, tag="ps")
            for n in range(NB // 512):
                mm(g2p[:, 512 * n:512 * (n + 1)],
                   wr[0:64, _C_WW2T:_C_WW2T + 128],
                   g1[:, 512 * n:512 * (n + 1)])
            g2 = wpool.tile([128, NB], F32R)
            nc.scalar.activation(g2[:], g2p[:], AF.Relu,
                                 bias=wf[:, _C_WB2:_C_WB2 + 1])

            wlp = ppool.tile([10, NB], F32, tag="ps")
            for n in range(NB // 512):
                mm(wlp[:, 512 * n:512 * (n + 1)],
                   wr[:, _C_WW3T:_C_WW3T + 10],
                   g2[:, 512 * n:512 * (n + 1)])
            e = wpool.tile([10, NB], F32R)
            nc.scalar.activation(e[:], wlp[:], AF.Exp,
                                 bias=wf[0:10, _C_WB3:_C_WB3 + 1])

            # --- pairing: cp_mean = P20.T @ cp -> [10, NB]
            pairp = ppool.tile([10, NB], F32, tag="ps")
            for n in range(NB // 512):
                sl = slice(512 * n, 512 * (n + 1))
                mm(pairp[:, sl], wr[0:20, _C_P20:_C_P20 + 10], cp[:, sl])

            # num lhsT rows: e * cp_mean  (DVE, psum x sbuf)
            wcpmN = wpool.tile([10, NB], F32R)
            nc.vector.tensor_mul(wcpmN[:], pairp[:], e[:].bitcast(F32))

            # --- output M-blocks (den emitted first so recip(j+1) can
            # overlap mul(j) with only 2 psum slots)
            for j in range(NB // 128):
                bsl = slice(128 * j, 128 * (j + 1))
                denp = qpool.tile([128, F], F32, tag="out")
                for h in range(F // 512):
                    fsl = slice(512 * h, 512 * (h + 1))
                    mm(denp[:, fsl], e[:, bsl], bt[:, fsl])
                nump = qpool.tile([128, F], F32, tag="out")
                for h in range(F // 512):
                    fsl = slice(512 * h, 512 * (h + 1))
                    mm(nump[:, fsl], wcpmN[:, bsl], bt[:, fsl])
                r = rpool.tile([128, F], F32)
                nc.vector.reciprocal_approx_fast(out=r[:], in_=denp[:])
                o = opool.tile([128, F], F32)
                if (blk, j) in GP_MUL:
                    numS = npool.tile([128, F], F32)
                    nc.scalar.copy(numS[:], nump[:])
                    nc.gpsimd.tensor_mul(o[:], numS[:], r[:])
                else:
                    nc.vector.tensor_mul(o[:], nump[:], r[:])
                dma_eng = nc.gpsimd if (blk, j) in GP_DMA else nc.sync
                dma_eng.dma_start(out[x0 + 128 * j:x0 + 128 * (j + 1), :],
                                  o[:])
            x0 += NB

    nc.compile()
    return nc


def host_consts(cp_w1, cp_b1, cp_w2, cp_b2, cp_w3, cp_b3,
                w_w1, w_b1, w_w2, w_b2, w_w3, w_b3):
    basis = basis_matrix()                     # [F, P]

    wr = np.zeros((128, C_R), np.float32)
    wr[:, _C_W1T:_C_W1T + 128] = cp_w1.T       # [128,128]
    wr[:, _C_W2T:_C_W2T + 256] = cp_w2.T       # [128,256]
    w3t = cp_w3.T                              # [256,20]
    wr[:, _C_W3T:_C_W3T + 20] = w3t[0:128]
    wr[:, _C_W3T + 20:_C_W3T + 40] = w3t[128:256]
    wr[:, _C_WW1T:_C_WW1T + 64] = w_w1.T       # [128,64]
    wr[0:64, _C_WW2T:_C_WW2T + 128] = w_w2.T   # [64,128]
    wr[:, _C_WW3T:_C_WW3T + 10] = w_w3.T       # [128,10]
    p20 = np.zeros((20, 10), np.float32)
    for p in range(P):
        p20[2 * p, p] = 0.5
        p20[2 * p + 1, p] = 0.5
    wr[0:20, _C_P20:_C_P20 + 10] = p20
    wr = round_f32r(wr)

    bt = round_f32r(np.ascontiguousarray(basis.T))   # [P, F]

    wf = np.zeros((128, C_F), np.float32)
    wf[:, _C_ID:_C_ID + 128] = np.eye(128, dtype=np.float32)
    wf[:, _C_B1] = cp_b1
    wf[:, _C_B2A] = cp_b2[0:128]
    wf[:, _C_B2B] = cp_b2[128:256]
    wf[0:20, _C_B3] = cp_b3
    wf[0:64, _C_WB1] = w_b1
    wf[:, _C_WB2] = w_b2
    wf[0:10, _C_WB3] = w_b3
    return wr, bt, wf


_NC_CACHE = None


def get_program():
    global _NC_CACHE
    if _NC_CACHE is None:
        _NC_CACHE = build_program()
    return _NC_CACHE


def kernel(x, cp_w1, cp_b1, cp_w2, cp_b2, cp_w3, cp_b3,
           w_w1, w_b1, w_w2, w_b2, w_w3, w_b3, _return_raw=False):
    x = np.asarray(x, np.float32)
    wr, bt, wf = host_consts(
        np.asarray(cp_w1, np.float32), np.asarray(cp_b1, np.float32),
        np.asarray(cp_w2, np.float32), np.asarray(cp_b2, np.float32),
        np.asarray(cp_w3, np.float32), np.asarray(cp_b3, np.float32),
        np.asarray(w_w1, np.float32), np.asarray(w_b1, np.float32),
        np.asarray(w_w2, np.float32), np.asarray(w_b2, np.float32),
        np.asarray(w_w3, np.float32), np.asarray(w_b3, np.float32))

    nc = get_program()
    in_maps = [
        {"x": np.ascontiguousarray(x[i * BC:(i + 1) * BC]),
         "wr": wr, "bt": bt, "wf": wf}
        for i in range(NCORES)
    ]
    res = run_bass_kernel_spmd(nc, in_maps, list(range(NCORES)))
    outs = [res.results[i]["out"] for i in range(NCORES)]
    full = np.concatenate(outs, axis=0)
    if _return_raw:
        return full, res
    return full



# revision 41
# speedup vs baseline: 1.4488x; 1.4488x over previous
"""Trainium2 Bass kernel for nn_CPWGenerator (B=16384, D=128, P=10, F=1024).

Data-parallel over batch across 8 NeuronCores (2048 rows/core). Per core:
  - feature-major 3-layer MLPs (control-point head + weight head)
  - softmax denominator cancels: out = num/den with raw e = exp(logits)
    (scale-invariant; reference's +1e-8 eps shifts results by <1.1e-7)
  - RATIO INTERPOLATION: out(t) = N(t)/D(t) is a ratio of Gaussian
    mixtures with sigma = 0.1 in t-units. Evaluate the ratio at S=128
    uniform sample points (unnormalized basis -- normalization cancels),
    then cubic-Lagrange interpolate to the F=1024 feature grid with
    [S,F] fp16 matmuls per 128-row batch chunk. Interp error ~1e-4;
    fp16 operand rounding ~5e-4 -- budget is 2e-2.
  - output written to HBM as fp16 (values are convex combos of tanh
    outputs, |out| <= 1), host converts to fp32: halves out-DMA bytes.
  - wl matmul uses duplicated W3w columns so exp() lands directly on a
    [20,NB] e_dup tile; the pairing matmul is folded into the sample
    basis (rows 2p/2p+1 carry 0.5*phi_p).
  - GPSIMD cannot touch PSUM on TRN2, so all PSUM evacuations run on
    ACT/DVE; GPSIMD handles SBUF-only work (ecp, fp32->fp16 converts
    behind a DVE uint64-bitcast PSUM copy that halves DVE column count).
Matmuls run as float32r (fp32 storage, 11-bit-mantissa operand rounding,
exact fp32 accumulation) at full PE rate; interp matmuls run fp16.
"""
import sys
if "/opt/trn_rl_repo" not in sys.path:
    sys.path.insert(0, "/opt/trn_rl_repo")

from contextlib import ExitStack

import numpy as np

import concourse.bacc as bacc
import concourse.mybir as mybir
import concourse.tile as tile
from concourse.bass_utils import run_bass_kernel_spmd

F32 = mybir.dt.float32
F32R = mybir.dt.float32r
F16 = mybir.dt.float16
U64 = mybir.dt.uint64
AF = mybir.ActivationFunctionType
ALU = mybir.AluOpType

# problem shapes (hardcoded per contest contract)
B, D, P, F = 16384, 128, 10, 1024
NCORES = 8
BC = B // NCORES          # rows per core = 2048
NB = 512                  # batch block
NBLK = BC // NB           # 4 blocks
S = 128                   # ratio sample count
EPS = 1e-8

# f32r const blob column offsets (layer-1 weights first: they ride a
# small early DMA that ungates the first matmuls)
_C_W1T = 0            # [128 x 128]
_C_WW1T = 128         # [128 x 64]
C_R1 = 192            # first-DMA split point
_C_W2T = 192          # [128 x 256]
_C_W3T = 448          # [128 x 40]  (W3Ta | W3Tb, 20 cols each)
_C_WW2T = 488         # [64  x 128]
_C_WW3D = 616         # [128 x 20]  w MLP final, columns duplicated per pair
_C_BTSP = 636         # [20  x S]   0.5 * phi_p(t_s) on rows 2p, 2p+1
C_R = 636 + S

# fp32 const blob columns (biases)
_C_B1 = 0
_C_B2A = 1
_C_B2B = 2
_C_B3 = 3
_C_WB1 = 4
_C_WB2 = 5
_C_WB3D = 6           # w MLP final bias duplicated per pair (20 rows)
C_F = 7


def round_f32r(x: np.ndarray) -> np.ndarray:
    """fp32 -> fp32r rounding (keep 11 explicit mantissa bits, RNE).
    Matches TRN2 hardware exactly (validated on device)."""
    u = np.ascontiguousarray(x, dtype=np.float32).view(np.uint32)
    keep = np.uint32(0xFFFFF000)
    half = np.uint32(0x800)
    lsb = (u >> np.uint32(12)) & np.uint32(1)
    r = (u + half - np.uint32(1) + lsb) & keep
    return r.view(np.float32)


def sample_basis() -> np.ndarray:
    """[20, S]: rows 2p and 2p+1 hold 0.5*phi_p(t_s), unnormalized
    Gaussian basis at the S uniform sample points (normalization cancels
    in the num/den ratio; the 0.5 pair split computes cp_mean)."""
    ts = np.arange(S, dtype=np.float64) / (S - 1)
    centers = np.arange(P, dtype=np.float64) / (P - 1)
    sigma = 1.0 / P
    phi = np.exp(-((ts[None, :] - centers[:, None]) ** 2)
                 / (2.0 * sigma * sigma))          # [P, S]
    bt = np.zeros((2 * P, S), np.float64)
    bt[0::2] = 0.5 * phi
    bt[1::2] = 0.5 * phi
    return bt.astype(np.float32)


def interp_matrix() -> np.ndarray:
    """[S, F] cubic-Lagrange interpolation matrix from the S uniform
    sample grid to the F uniform feature grid (both span [0,1])."""
    ts = np.arange(S, dtype=np.float64) / (S - 1)
    M = np.zeros((S, F), np.float64)
    for f in range(F):
        tf = f / (F - 1)
        j = int(np.floor(tf * (S - 1)))
        j0 = min(max(j - 1, 0), S - 4)
        xs = ts[j0:j0 + 4]
        for a in range(4):
            L = 1.0
            for b_ in range(4):
                if a != b_:
                    L *= (tf - xs[b_]) / (xs[a] - xs[b_])
            M[j0 + a, f] = L
    return M.astype(np.float16)


# engine assignment config (sweepable)
CFG = {
    # engines for the relu/copy evacuations (PSUM readers: act/dve only)
    "xt": "dve", "h1": "dve", "h2a": "act", "h2b": "act",
    "g1": "dve", "g2": "act",
    "ecp": "pool",
    # per-half interp evac modes, cycled: "act"/"dve" = direct fp16 copy;
    # "u64" = DVE uint64 psum copy (half cols) + Pool sbuf fp16 convert
    "evac": ["dve", "act", "act", "dve", "act", "dve", "act", "act"],
    "order": None,
    "skew": 3.0,
}


def build_program():
    nc = bacc.Bacc()
    x_in = nc.declare_dram_parameter("x", [BC, D], F32R, isOutput=False)
    wr_in = nc.declare_dram_parameter("wr", [128, C_R], F32R, isOutput=False)
    wf_in = nc.declare_dram_parameter("wf", [128, C_F], F32, isOutput=False)
    im_in = nc.declare_dram_parameter("im", [S, F], F16, isOutput=False)
    out = nc.declare_dram_parameter("out", [BC, F], F16, isOutput=True)

    with tile.TileContext(nc) as tc, ExitStack() as ctx:
        cpool = ctx.enter_context(tc.tile_pool(name="const", bufs=1))
        _wnames = ["xt", "h1", "h2a", "h2b", "cp", "g1", "g2", "eec",
                   "rs", "outs"]
        vp = {n: ctx.enter_context(tc.tile_pool(name=n, bufs=2))
              for n in _wnames}
        opool = ctx.enter_context(tc.tile_pool(name="outp", bufs=8))
        spool = ctx.enter_context(tc.tile_pool(name="stg", bufs=4))
        pp2 = [ctx.enter_context(tc.tile_pool(name=f"psum{i}", bufs=2,
                                              space="PSUM"))
               for i in range(2)]
        qpool = ctx.enter_context(tc.tile_pool(name="psumo", bufs=4,
                                               space="PSUM"))

        identt = cpool.tile([128, 128], F32R)
        wr = cpool.tile([128, C_R], F32R)
        wf = cpool.tile([128, C_F], F32)
        im = cpool.tile([S, F], F16)
        xb = [cpool.tile([128, NB], F32R, name=f"xb{i}")
              for i in range(NBLK)]

        def x_dma(blk):
            nc.sync.dma_start(
                xb[blk][:].rearrange("p (c d) -> p c d", c=NB // 128),
                x_in[blk * NB:(blk + 1) * NB, :].rearrange(
                    "(c p) d -> p c d", p=128),
            )

        # identity built on-device (no DMA dependency for the transposes)
        nc.gpsimd.memset(identt[:].bitcast(F32), 1.0)
        nc.gpsimd.affine_select(identt[:], identt[:], [[1, 128]],
                                mybir.AluOpType.is_equal, 0.0,
                                base=0, channel_multiplier=-1)
        # dummy table-func activation: hoists the 1.28us ACT table load
        # off the critical chain (it otherwise lands right before exp(0))
        scr = cpool.tile([1, 1], F32)
        nc.gpsimd.memset(scr[:], 0.0)
        nc.scalar.activation(scr[:], scr[:], AF.Relu)

        # in-DMA order tuned for pipeline fill: x block 0, layer-1
        # weights, biases, remaining weights, the rest
        x_dma(0)
        nc.sync.dma_start(wr[:, 0:C_R1], wr_in[:, 0:C_R1])
        nc.sync.dma_start(wf[:], wf_in[:])
        nc.sync.dma_start(wr[:, C_R1:C_R], wr_in[:, C_R1:C_R])
        x_dma(1)
        x_dma(2)
        nc.sync.dma_start(im[:], im_in[:])
        for blk in range(3, NBLK):
            x_dma(blk)

        ident = identt[:]
        ENG = {"act": nc.scalar, "dve": nc.vector, "pool": nc.gpsimd}

        def mm(out_ap, lhsT, rhs, start=True, stop=True):
            nc.tensor.matmul(out_ap, lhsT, rhs, start=start, stop=stop)

        # psum -> sbuf evacuation with relu+bias: act/dve direct, or
        # "u64p" = DVE uint64 raw copy (half cols) + Pool relu (SBUF-only)
        def evac_relu(name, dst, src, bias_col, rows=128):
            eng = CFG[name]
            if eng == "act":
                nc.scalar.activation(dst[:], src, AF.Relu,
                                     bias=wf[0:rows, bias_col:bias_col + 1])
            elif eng == "u64p":
                stg = spool.tile([rows, NB], F32, name=f"stg_{name}")
                nc.vector.tensor_copy(stg[:].bitcast(U64), src.bitcast(U64))
                nc.gpsimd.tensor_scalar(
                    dst[:], stg[:], wf[0:rows, bias_col:bias_col + 1],
                    0.0, ALU.add, ALU.max)
            else:
                ENG[eng].tensor_scalar(
                    dst[:], src, wf[0:rows, bias_col:bias_col + 1],
                    0.0, ALU.add, ALU.max)

        outs_t = [None] * NBLK
        state = [dict() for _ in range(NBLK)]

        def front_atoms(blk):
            """Staged atoms: transpose + MLPs + ratio samples -> outs."""
            ppool = pp2[blk % 2]
            st = state[blk]

            def a_xt():
                xtp = ppool.tile([128, NB], F32R, tag="ps")
                for c in range(NB // 128):
                    nc.tensor.matmul(
                        xtp[:, 128 * c:128 * (c + 1)],
                        xb[blk][:, 128 * c:128 * (c + 1)],
                        ident,
                        is_transpose=True,
                        start=(c == 0),
                        stop=(c == NB // 128 - 1),
                    )
                xt = vp["xt"].tile([128, NB], F32R)
                if CFG["xt"] == "act":
                    nc.scalar.activation(xt[:], xtp[:].bitcast(F32), AF.Copy)
                elif CFG["xt"] == "u64p":
                    stg = spool.tile([128, NB], F32, name="stg_xt")
                    nc.vector.tensor_copy(stg[:].bitcast(U64),
                                          xtp[:].bitcast(U64))
                    nc.gpsimd.tensor_copy(xt[:], stg[:])
                else:
                    nc.vector.tensor_copy(xt[:], xtp[:].bitcast(F32))
                st["xt"] = xt

            def a_g1():
                g1p = ppool.tile([64, NB], F32, tag="ps")
                mm(g1p[:], wr[:, _C_WW1T:_C_WW1T + 64], st["xt"][:])
                g1 = vp["g1"].tile([64, NB], F32R)
                evac_relu("g1", g1, g1p[:], _C_WB1, rows=64)
                st["g1"] = g1

            def a_g2():
                g2p = ppool.tile([128, NB], F32, tag="ps")
                mm(g2p[:], wr[0:64, _C_WW2T:_C_WW2T + 128], st["g1"][:])
                g2 = vp["g2"].tile([128, NB], F32R)
                evac_relu("g2", g2, g2p[:], _C_WB2)
                st["g2"] = g2

            def a_exp():
                wlp = ppool.tile([20, NB], F32, tag="ps")
                mm(wlp[:], wr[:, _C_WW3D:_C_WW3D + 20], st["g2"][:])
                eec = vp["eec"].tile([20, 2 * NB], F32R)
                nc.scalar.activation(eec[:, 0:NB], wlp[:], AF.Exp,
                                     bias=wf[0:20, _C_WB3D:_C_WB3D + 1])
                st["eec"] = eec

            def a_h1():
                h1p = ppool.tile([128, NB], F32, tag="ps")
                mm(h1p[:], wr[:, _C_W1T:_C_W1T + 128], st["xt"][:])
                h1 = vp["h1"].tile([128, NB], F32R)
                evac_relu("h1", h1, h1p[:], _C_B1)
                st["h1"] = h1

            def a_h2a():
                h2pa = ppool.tile([128, NB], F32, tag="ps")
                mm(h2pa[:], wr[:, _C_W2T:_C_W2T + 128], st["h1"][:])
                h2a = vp["h2a"].tile([128, NB], F32R)
                evac_relu("h2a", h2a, h2pa[:], _C_B2A)
                st["h2a"] = h2a

            def a_h2b():
                h2pb = ppool.tile([128, NB], F32, tag="ps")
                mm(h2pb[:], wr[:, _C_W2T + 128:_C_W2T + 256], st["h1"][:])
                h2b = vp["h2b"].tile([128, NB], F32R)
                evac_relu("h2b", h2b, h2pb[:], _C_B2B)
                st["h2b"] = h2b

            def a_tanh():
                cpp = ppool.tile([20, NB], F32, tag="ps")
                mm(cpp[:], wr[:, _C_W3T:_C_W3T + 20], st["h2a"][:],
                   stop=False)
                mm(cpp[:], wr[:, _C_W3T + 20:_C_W3T + 40], st["h2b"][:],
                   start=False, stop=True)
                cp = vp["cp"].tile([20, NB], F32R)
                nc.scalar.activation(cp[:], cpp[:], AF.Tanh,
                                     bias=wf[0:20, _C_B3:_C_B3 + 1])
                st["cp"] = cp

            def a_ecp():
                eec = st["eec"]
                ENG[CFG["ecp"]].tensor_mul(
                    eec[:, NB:2 * NB], st["cp"][:].bitcast(F32),
                    eec[:, 0:NB].bitcast(F32))

            def a_ratio():
                eec = st["eec"]
                den = qpool.tile([S, NB], F32, tag="out")
                mm(den[:], wr[0:20, _C_BTSP:_C_BTSP + S], eec[:, 0:NB])
                num = qpool.tile([S, NB], F32, tag="out")
                mm(num[:], wr[0:20, _C_BTSP:_C_BTSP + S], eec[:, NB:2 * NB])
                rs = vp["rs"].tile([S, NB], F32)
                nc.vector.reciprocal_approx_fast(out=rs[:], in_=den[:])
                outs = vp["outs"].tile([S, NB], F16)
                nc.vector.tensor_mul(outs[:], num[:], rs[:])
                outs_t[blk] = outs

            return [(0.0, a_xt), (1.0, a_g1), (2.0, a_g2), (3.0, a_exp),
                    (1.5, a_h1), (2.5, a_h2a), (2.7, a_h2b), (3.5, a_tanh),
                    (4.0, a_ecp), (4.6, a_ratio)]

        evac_rr = [0]

        def back_atoms(blk):
            """One atom per 128-row chunk: 2 interp mms + evacs + DMA."""
            atoms = []
            for j in range(NB // 128):
                def a_chunk(j=j):
                    outs = outs_t[blk]
                    obuf = opool.tile([128, F], F16)
                    for h in range(F // 512):
                        fsl = slice(512 * h, 512 * (h + 1))
                        obp = qpool.tile([128, 512], F32, tag="out")
                        mm(obp[:], outs[:, 128 * j:128 * (j + 1)],
                           im[:, fsl])
                        mode = CFG["evac"][evac_rr[0] % len(CFG["evac"])]
                        evac_rr[0] += 1
                        if mode == "act":
                            nc.scalar.copy(obuf[:, fsl], obp[:])
                        else:
                            nc.vector.tensor_copy(obuf[:, fsl], obp[:])
                    r0 = blk * NB + j * 128
                    nc.sync.dma_start(out[r0:r0 + 128, :], obuf[:])
                atoms.append((5.6 + 0.5 * j, a_chunk))
            return atoms

        skew = CFG.get("skew")
        if skew is None:
            # block-sequential emission per CFG order string
            fronts = [front_atoms(k) for k in range(NBLK)]
            backs = [back_atoms(k) for k in range(NBLK)]
            fi, bi = 0, 0
            order = CFG["order"] or ("AA" + "BA" * (NBLK - 2) + "BB")
            for ch in order:
                if ch == "A":
                    for _, fn in fronts[fi]:
                        fn()
                    fi += 1
                else:
                    for _, fn in backs[bi]:
                        fn()
                    bi += 1
            assert fi == NBLK and bi == NBLK
        else:
            # wavefront emission: priority = stage + blk * skew
            allatoms = []
            for k in range(NBLK):
                for s, fn in front_atoms(k) + back_atoms(k):
                    allatoms.append((s + k * skew, k, s, fn))
            allatoms.sort(key=lambda t: (t[0], t[1], t[2]))
            for _, _, _, fn in allatoms:
                fn()

    nc.compile()
    return nc


def host_consts(cp_w1, cp_b1, cp_w2, cp_b2, cp_w3, cp_b3,
                w_w1, w_b1, w_w2, w_b2, w_w3, w_b3):
    wr = np.zeros((128, C_R), np.float32)
    wr[:, _C_W1T:_C_W1T + 128] = cp_w1.T       # [128,128]
    wr[:, _C_W2T:_C_W2T + 256] = cp_w2.T       # [128,256]
    w3t = cp_w3.T                              # [256,20]
    wr[:, _C_W3T:_C_W3T + 20] = w3t[0:128]
    wr[:, _C_W3T + 20:_C_W3T + 40] = w3t[128:256]
    wr[:, _C_WW1T:_C_WW1T + 64] = w_w1.T       # [128,64]
    wr[0:64, _C_WW2T:_C_WW2T + 128] = w_w2.T   # [64,128]
    w3w = w_w3.T                               # [128,10]
    wr[:, _C_WW3D + 0:_C_WW3D + 20:2] = w3w
    wr[:, _C_WW3D + 1:_C_WW3D + 20:2] = w3w
    wr[0:20, _C_BTSP:_C_BTSP + S] = sample_basis()
    wr = round_f32r(wr)

    wf = np.zeros((128, C_F), np.float32)
    wf[:, _C_B1] = cp_b1
    wf[:, _C_B2A] = cp_b2[0:128]
    wf[:, _C_B2B] = cp_b2[128:256]
    wf[0:20, _C_B3] = cp_b3
    wf[0:64, _C_WB1] = w_b1
    wf[:, _C_WB2] = w_b2
    wf[0:20, _C_WB3D:_C_WB3D + 1] = np.repeat(w_b3, 2)[:, None]
    im = interp_matrix()
    return wr, wf, im


_NC_CACHE = None


def get_program():
    global _NC_CACHE
    if _NC_CACHE is None:
        _NC_CACHE = build_program()
    return _NC_CACHE


def kernel(x, cp_w1, cp_b1, cp_w2, cp_b2, cp_w3, cp_b3,
           w_w1, w_b1, w_w2, w_b2, w_w3, w_b3, _return_raw=False):
    x = np.asarray(x, np.float32)
    wr, wf, im = host_consts(
        np.asarray(cp_w1, np.float32), np.asarray(cp_b1, np.float32),
        np.asarray(cp_w2, np.float32), np.asarray(cp_b2, np.float32),
        np.asarray(cp_w3, np.float32), np.asarray(cp_b3, np.float32),
        np.asarray(w_w1, np.float32), np.asarray(w_b1, np.float32),
        np.asarray(w_w2, np.float32), np.asarray(w_b2, np.float32),
        np.asarray(w_w3, np.float32), np.asarray(w_b3, np.float32))

    nc = get_program()
    in_maps = [
        {"x": np.ascontiguousarray(x[i * BC:(i + 1) * BC]),
         "wr": wr, "wf": wf, "im": im}
        for i in range(NCORES)
    ]
    res = run_bass_kernel_spmd(nc, in_maps, list(range(NCORES)))
    outs = [res.results[i]["out"] for i in range(NCORES)]
    full = np.concatenate(outs, axis=0).astype(np.float32)
    if _return_raw:
        return full, res
    return full


# revision 44
# speedup vs baseline: 1.4617x; 1.0089x over previous
"""Trainium2 Bass kernel for nn_CPWGenerator (B=16384, D=128, P=10, F=1024).

Data-parallel over batch across 8 NeuronCores (2048 rows/core). Per core:
  - feature-major 3-layer MLPs (control-point head + weight head)
  - softmax denominator cancels: out = num/den with raw e = exp(logits)
    (scale-invariant; reference's +1e-8 eps shifts results by <1.1e-7)
  - RATIO INTERPOLATION: out(t) = N(t)/D(t) is a ratio of Gaussian
    mixtures with sigma = 0.1 in t-units. Evaluate the ratio at S=128
    uniform sample points (unnormalized basis -- normalization cancels),
    then cubic-Lagrange interpolate to the F=1024 feature grid with
    [S,F] fp16 matmuls per 128-row batch chunk. Interp error ~1e-4;
    fp16 operand rounding ~5e-4 -- budget is 2e-2.
  - output written to HBM as fp16 (values are convex combos of tanh
    outputs, |out| <= 1), host converts to fp32: halves out-DMA bytes.
  - wl matmul uses duplicated W3w columns so exp() lands directly on a
    [20,NB] e_dup tile; the pairing matmul is folded into the sample
    basis (rows 2p/2p+1 carry 0.5*phi_p).
  - GPSIMD cannot touch PSUM on TRN2, so all PSUM evacuations run on
    ACT/DVE; GPSIMD handles SBUF-only work (ecp, fp32->fp16 converts
    behind a DVE uint64-bitcast PSUM copy that halves DVE column count).
Matmuls run as float32r (fp32 storage, 11-bit-mantissa operand rounding,
exact fp32 accumulation) at full PE rate; interp matmuls run fp16.
"""
import sys
if "/opt/trn_rl_repo" not in sys.path:
    sys.path.insert(0, "/opt/trn_rl_repo")

from contextlib import ExitStack

import numpy as np

import concourse.bacc as bacc
import concourse.mybir as mybir
import concourse.tile as tile
from concourse.bass_utils import run_bass_kernel_spmd

F32 = mybir.dt.float32
F32R = mybir.dt.float32r
F16 = mybir.dt.float16
U64 = mybir.dt.uint64
AF = mybir.ActivationFunctionType
ALU = mybir.AluOpType

# problem shapes (hardcoded per contest contract)
B, D, P, F = 16384, 128, 10, 1024
NCORES = 8
BC = B // NCORES          # rows per core = 2048
NB = 512                  # batch block
NBLK = BC // NB           # 4 blocks
S = 128                   # ratio sample count
EPS = 1e-8

# f32r const blob column offsets (layer-1 weights first: they ride a
# small early DMA that ungates the first matmuls)
_C_W1T = 0            # [128 x 128]
_C_WW1T = 128         # [128 x 64]
C_R1 = 192            # first-DMA split point
_C_W2T = 192          # [128 x 256]
_C_W3T = 448          # [128 x 40]  (W3Ta | W3Tb, 20 cols each)
_C_WW2T = 488         # [64  x 128]
_C_WW3D = 616         # [128 x 20]  w MLP final, columns duplicated per pair
_C_BTSP = 636         # [20  x S]   0.5 * phi_p(t_s) on rows 2p, 2p+1
C_R = 636 + S

# fp32 const blob columns (biases)
_C_B1 = 0
_C_B2A = 1
_C_B2B = 2
_C_B3 = 3
_C_WB1 = 4
_C_WB2 = 5
_C_WB3D = 6           # w MLP final bias duplicated per pair (20 rows)
C_F = 7


def round_f32r(x: np.ndarray) -> np.ndarray:
    """fp32 -> fp32r rounding (keep 11 explicit mantissa bits, RNE).
    Matches TRN2 hardware exactly (validated on device)."""
    u = np.ascontiguousarray(x, dtype=np.float32).view(np.uint32)
    keep = np.uint32(0xFFFFF000)
    half = np.uint32(0x800)
    lsb = (u >> np.uint32(12)) & np.uint32(1)
    r = (u + half - np.uint32(1) + lsb) & keep
    return r.view(np.float32)


def sample_basis() -> np.ndarray:
    """[20, S]: rows 2p and 2p+1 hold 0.5*phi_p(t_s), unnormalized
    Gaussian basis at the S uniform sample points (normalization cancels
    in the num/den ratio; the 0.5 pair split computes cp_mean)."""
    ts = np.arange(S, dtype=np.float64) / (S - 1)
    centers = np.arange(P, dtype=np.float64) / (P - 1)
    sigma = 1.0 / P
    phi = np.exp(-((ts[None, :] - centers[:, None]) ** 2)
                 / (2.0 * sigma * sigma))          # [P, S]
    bt = np.zeros((2 * P, S), np.float64)
    bt[0::2] = 0.5 * phi
    bt[1::2] = 0.5 * phi
    return bt.astype(np.float32)


def interp_matrix() -> np.ndarray:
    """[S, F] cubic-Lagrange interpolation matrix from the S uniform
    sample grid to the F uniform feature grid (both span [0,1])."""
    ts = np.arange(S, dtype=np.float64) / (S - 1)
    M = np.zeros((S, F), np.float64)
    for f in range(F):
        tf = f / (F - 1)
        j = int(np.floor(tf * (S - 1)))
        j0 = min(max(j - 1, 0), S - 4)
        xs = ts[j0:j0 + 4]
        for a in range(4):
            L = 1.0
            for b_ in range(4):
                if a != b_:
                    L *= (tf - xs[b_]) / (xs[a] - xs[b_])
            M[j0 + a, f] = L
    return M.astype(np.float16)


# engine assignment config (sweepable)
CFG = {
    # engines for the relu/copy evacuations (PSUM readers: act/dve only)
    "xt": "dve", "h1": "dve", "h2a": "act", "h2b": "act",
    "g1": "dve", "g2": "act",
    "ecp": "pool",
    # per-half interp evac modes, cycled: "act"/"dve" = direct fp16 copy;
    # "u64" = DVE uint64 psum copy (half cols) + Pool sbuf fp16 convert
    "evac": ["dve", "act", "act", "dve", "act", "dve", "act", "act"],
    "order": None,
    "skew": None,
    "waves": [0, 3, 6, 8],
}


def build_program():
    nc = bacc.Bacc()
    x_in = nc.declare_dram_parameter("x", [BC, D], F32R, isOutput=False)
    wr_in = nc.declare_dram_parameter("wr", [128, C_R], F32R, isOutput=False)
    wf_in = nc.declare_dram_parameter("wf", [128, C_F], F32, isOutput=False)
    im_in = nc.declare_dram_parameter("im", [S, F], F16, isOutput=False)
    out = nc.declare_dram_parameter("out", [BC, F], F16, isOutput=True)

    with tile.TileContext(nc) as tc, ExitStack() as ctx:
        cpool = ctx.enter_context(tc.tile_pool(name="const", bufs=1))
        _wnames = ["xt", "h1", "h2a", "h2b", "cp", "g1", "g2", "eec",
                   "rs", "outs"]
        vp = {n: ctx.enter_context(tc.tile_pool(name=n, bufs=2))
              for n in _wnames}
        opool = ctx.enter_context(tc.tile_pool(name="outp", bufs=8))
        spool = ctx.enter_context(tc.tile_pool(name="stg", bufs=4))
        pp2 = [ctx.enter_context(tc.tile_pool(name=f"psum{i}", bufs=2,
                                              space="PSUM"))
               for i in range(2)]
        qpool = ctx.enter_context(tc.tile_pool(name="psumo", bufs=4,
                                               space="PSUM"))

        identt = cpool.tile([128, 128], F32R)
        wr = cpool.tile([128, C_R], F32R)
        wf = cpool.tile([128, C_F], F32)
        im = cpool.tile([S, F], F16)
        xb = [cpool.tile([128, NB], F32R, name=f"xb{i}")
              for i in range(NBLK)]

        def x_dma(blk):
            nc.sync.dma_start(
                xb[blk][:].rearrange("p (c d) -> p c d", c=NB // 128),
                x_in[blk * NB:(blk + 1) * NB, :].rearrange(
                    "(c p) d -> p c d", p=128),
            )

        # identity built on-device (no DMA dependency for the transposes)
        nc.gpsimd.memset(identt[:].bitcast(F32), 1.0)
        nc.gpsimd.affine_select(identt[:], identt[:], [[1, 128]],
                                mybir.AluOpType.is_equal, 0.0,
                                base=0, channel_multiplier=-1)
        # dummy table-func activation: hoists the 1.28us ACT table load
        # off the critical chain (it otherwise lands right before exp(0))
        scr = cpool.tile([1, 1], F32)
        nc.gpsimd.memset(scr[:], 0.0)
        nc.scalar.activation(scr[:], scr[:], AF.Relu)

        # in-DMA order tuned for pipeline fill: x block 0, layer-1
        # weights, biases, remaining weights, the rest
        x_dma(0)
        nc.sync.dma_start(wr[:, 0:C_R1], wr_in[:, 0:C_R1])
        nc.sync.dma_start(wf[:], wf_in[:])
        nc.sync.dma_start(wr[:, C_R1:C_R], wr_in[:, C_R1:C_R])
        x_dma(1)
        x_dma(2)
        nc.sync.dma_start(im[:], im_in[:])
        for blk in range(3, NBLK):
            x_dma(blk)

        ident = identt[:]
        ENG = {"act": nc.scalar, "dve": nc.vector, "pool": nc.gpsimd}

        def mm(out_ap, lhsT, rhs, start=True, stop=True):
            nc.tensor.matmul(out_ap, lhsT, rhs, start=start, stop=stop)

        # psum -> sbuf evacuation with relu+bias: act/dve direct, or
        # "u64p" = DVE uint64 raw copy (half cols) + Pool relu (SBUF-only)
        def evac_relu(name, dst, src, bias_col, rows=128):
            eng = CFG[name]
            if eng == "act":
                nc.scalar.activation(dst[:], src, AF.Relu,
                                     bias=wf[0:rows, bias_col:bias_col + 1])
            elif eng == "u64p":
                stg = spool.tile([rows, NB], F32, name=f"stg_{name}")
                nc.vector.tensor_copy(stg[:].bitcast(U64), src.bitcast(U64))
                nc.gpsimd.tensor_scalar(
                    dst[:], stg[:], wf[0:rows, bias_col:bias_col + 1],
                    0.0, ALU.add, ALU.max)
            else:
                ENG[eng].tensor_scalar(
                    dst[:], src, wf[0:rows, bias_col:bias_col + 1],
                    0.0, ALU.add, ALU.max)

        outs_t = [None] * NBLK
        state = [dict() for _ in range(NBLK)]

        def front_atoms(blk):
            """Staged atoms: transpose + MLPs + ratio samples -> outs."""
            ppool = pp2[blk % 2]
            st = state[blk]

            def a_xt():
                xtp = ppool.tile([128, NB], F32R, tag="ps")
                for c in range(NB // 128):
                    nc.tensor.matmul(
                        xtp[:, 128 * c:128 * (c + 1)],
                        xb[blk][:, 128 * c:128 * (c + 1)],
                        ident,
                        is_transpose=True,
                        start=(c == 0),
                        stop=(c == NB // 128 - 1),
                    )
                xt = vp["xt"].tile([128, NB], F32R)
                if CFG["xt"] == "act":
                    nc.scalar.activation(xt[:], xtp[:].bitcast(F32), AF.Copy)
                elif CFG["xt"] == "u64p":
                    stg = spool.tile([128, NB], F32, name="stg_xt")
                    nc.vector.tensor_copy(stg[:].bitcast(U64),
                                          xtp[:].bitcast(U64))
                    nc.gpsimd.tensor_copy(xt[:], stg[:])
                else:
                    nc.vector.tensor_copy(xt[:], xtp[:].bitcast(F32))
                st["xt"] = xt

            def a_g1():
                g1p = ppool.tile([64, NB], F32, tag="ps")
                mm(g1p[:], wr[:, _C_WW1T:_C_WW1T + 64], st["xt"][:])
                g1 = vp["g1"].tile([64, NB], F32R)
                evac_relu("g1", g1, g1p[:], _C_WB1, rows=64)
                st["g1"] = g1

            def a_g2():
                g2p = ppool.tile([128, NB], F32, tag="ps")
                mm(g2p[:], wr[0:64, _C_WW2T:_C_WW2T + 128], st["g1"][:])
                g2 = vp["g2"].tile([128, NB], F32R)
                evac_relu("g2", g2, g2p[:], _C_WB2)
                st["g2"] = g2

            def a_exp():
                wlp = ppool.tile([20, NB], F32, tag="ps")
                mm(wlp[:], wr[:, _C_WW3D:_C_WW3D + 20], st["g2"][:])
                eec = vp["eec"].tile([20, 2 * NB], F32R)
                nc.scalar.activation(eec[:, 0:NB], wlp[:], AF.Exp,
                                     bias=wf[0:20, _C_WB3D:_C_WB3D + 1])
                st["eec"] = eec

            def a_h1():
                h1p = ppool.tile([128, NB], F32, tag="ps")
                mm(h1p[:], wr[:, _C_W1T:_C_W1T + 128], st["xt"][:])
                h1 = vp["h1"].tile([128, NB], F32R)
                evac_relu("h1", h1, h1p[:], _C_B1)
                st["h1"] = h1

            def a_h2a():
                h2pa = ppool.tile([128, NB], F32, tag="ps")
                mm(h2pa[:], wr[:, _C_W2T:_C_W2T + 128], st["h1"][:])
                h2a = vp["h2a"].tile([128, NB], F32R)
                evac_relu("h2a", h2a, h2pa[:], _C_B2A)
                st["h2a"] = h2a

            def a_h2b():
                h2pb = ppool.tile([128, NB], F32, tag="ps")
                mm(h2pb[:], wr[:, _C_W2T + 128:_C_W2T + 256], st["h1"][:])
                h2b = vp["h2b"].tile([128, NB], F32R)
                evac_relu("h2b", h2b, h2pb[:], _C_B2B)
                st["h2b"] = h2b

            def a_tanh():
                cpp = ppool.tile([20, NB], F32, tag="ps")
                mm(cpp[:], wr[:, _C_W3T:_C_W3T + 20], st["h2a"][:],
                   stop=False)
                mm(cpp[:], wr[:, _C_W3T + 20:_C_W3T + 40], st["h2b"][:],
                   start=False, stop=True)
                cp = vp["cp"].tile([20, NB], F32R)
                nc.scalar.activation(cp[:], cpp[:], AF.Tanh,
                                     bias=wf[0:20, _C_B3:_C_B3 + 1])
                st["cp"] = cp

            def a_ecp():
                eec = st["eec"]
                ENG[CFG["ecp"]].tensor_mul(
                    eec[:, NB:2 * NB], st["cp"][:].bitcast(F32),
                    eec[:, 0:NB].bitcast(F32))

            def a_ratio():
                eec = st["eec"]
                den = qpool.tile([S, NB], F32, tag="out")
                mm(den[:], wr[0:20, _C_BTSP:_C_BTSP + S], eec[:, 0:NB])
                num = qpool.tile([S, NB], F32, tag="out")
                mm(num[:], wr[0:20, _C_BTSP:_C_BTSP + S], eec[:, NB:2 * NB])
                rs = vp["rs"].tile([S, NB], F32)
                nc.vector.reciprocal_approx_fast(out=rs[:], in_=den[:])
                outs = vp["outs"].tile([S, NB], F16)
                nc.vector.tensor_mul(outs[:], num[:], rs[:])
                outs_t[blk] = outs

            return [(0.0, a_xt), (1.0, a_g1), (2.0, a_g2), (3.0, a_exp),
                    (1.5, a_h1), (2.5, a_h2a), (2.7, a_h2b), (3.5, a_tanh),
                    (4.0, a_ecp), (4.6, a_ratio)]

        evac_rr = [0]

        def back_atoms(blk):
            """One atom per 128-row chunk: 2 interp mms + evacs + DMA."""
            atoms = []
            for j in range(NB // 128):
                def a_chunk(j=j):
                    outs = outs_t[blk]
                    obuf = opool.tile([128, F], F16)
                    for h in range(F // 512):
                        fsl = slice(512 * h, 512 * (h + 1))
                        obp = qpool.tile([128, 512], F32, tag="out")
                        mm(obp[:], outs[:, 128 * j:128 * (j + 1)],
                           im[:, fsl])
                        mode = CFG["evac"][evac_rr[0] % len(CFG["evac"])]
                        evac_rr[0] += 1
                        if mode == "act":
                            nc.scalar.copy(obuf[:, fsl], obp[:])
                        else:
                            nc.vector.tensor_copy(obuf[:, fsl], obp[:])
                    r0 = blk * NB + j * 128
                    nc.sync.dma_start(out[r0:r0 + 128, :], obuf[:])
                atoms.append((CFG.get("bstage", 5.6)
                              + CFG.get("bgap", 0.5) * j, a_chunk))
            return atoms

        waves = CFG.get("waves")
        skew = CFG.get("skew")
        if waves is not None:
            allatoms = []
            for k in range(NBLK):
                for s, fn in front_atoms(k) + back_atoms(k):
                    allatoms.append((s + waves[k], k, s, fn))
            allatoms.sort(key=lambda t: (t[0], t[1], t[2]))
            for _, _, _, fn in allatoms:
                fn()
            skew = "done"
        if skew == "done":
            pass
        elif skew is None:
            # block-sequential emission per CFG order string
            fronts = [front_atoms(k) for k in range(NBLK)]
            backs = [back_atoms(k) for k in range(NBLK)]
            fi, bi = 0, 0
            order = CFG["order"] or ("AA" + "BA" * (NBLK - 2) + "BB")
            for ch in order:
                if ch == "A":
                    for _, fn in fronts[fi]:
                        fn()
                    fi += 1
                else:
                    for _, fn in backs[bi]:
                        fn()
                    bi += 1
            assert fi == NBLK and bi == NBLK
        else:
            # wavefront emission: priority = stage + blk * skew
            allatoms = []
            for k in range(NBLK):
                for s, fn in front_atoms(k) + back_atoms(k):
                    allatoms.append((s + k * skew, k, s, fn))
            allatoms.sort(key=lambda t: (t[0], t[1], t[2]))
            for _, _, _, fn in allatoms:
                fn()

    nc.compile()
    return nc


def host_consts(cp_w1, cp_b1, cp_w2, cp_b2, cp_w3, cp_b3,
                w_w1, w_b1, w_w2, w_b2, w_w3, w_b3):
    wr = np.zeros((128, C_R), np.float32)
    wr[:, _C_W1T:_C_W1T + 128] = cp_w1.T       # [128,128]
    wr[:, _C_W2T:_C_W2T + 256] = cp_w2.T       # [128,256]
    w3t = cp_w3.T                              # [256,20]
    wr[:, _C_W3T:_C_W3T + 20] = w3t[0:128]
    wr[:, _C_W3T + 20:_C_W3T + 40] = w3t[128:256]
    wr[:, _C_WW1T:_C_WW1T + 64] = w_w1.T       # [128,64]
    wr[0:64, _C_WW2T:_C_WW2T + 128] = w_w2.T   # [64,128]
    w3w = w_w3.T                               # [128,10]
    wr[:, _C_WW3D + 0:_C_WW3D + 20:2] = w3w
    wr[:, _C_WW3D + 1:_C_WW3D + 20:2] = w3w
    wr[0:20, _C_BTSP:_C_BTSP + S] = sample_basis()
    wr = round_f32r(wr)

    wf = np.zeros((128, C_F), np.float32)
    wf[:, _C_B1] = cp_b1
    wf[:, _C_B2A] = cp_b2[0:128]
    wf[:, _C_B2B] = cp_b2[128:256]
    wf[0:20, _C_B3] = cp_b3
    wf[0:64, _C_WB1] = w_b1
    wf[:, _C_WB2] = w_b2
    wf[0:20, _C_WB3D:_C_WB3D + 1] = np.repeat(w_b3, 2)[:, None]
    im = interp_matrix()
    return wr, wf, im


_NC_CACHE = None


def get_program():
    global _NC_CACHE
    if _NC_CACHE is None:
        _NC_CACHE = build_program()
    return _NC_CACHE


def kernel(x, cp_w1, cp_b1, cp_w2, cp_b2, cp_w3, cp_b3,
           w_w1, w_b1, w_w2, w_b2, w_w3, w_b3, _return_raw=False):
    x = np.asarray(x, np.float32)
    wr, wf, im = host_consts(
        np.asarray(cp_w1, np.float32), np.asarray(cp_b1, np.float32),
        np.asarray(cp_w2, np.float32), np.asarray(cp_b2, np.float32),
        np.asarray(cp_w3, np.float32), np.asarray(cp_b3, np.float32),
        np.asarray(w_w1, np.float32), np.asarray(w_b1, np.float32),
        np.asarray(w_w2, np.float32), np.asarray(w_b2, np.float32),
        np.asarray(w_w3, np.float32), np.asarray(w_b3, np.float32))

    nc = get_program()
    in_maps = [
        {"x": np.ascontiguousarray(x[i * BC:(i + 1) * BC]),
         "wr": wr, "wf": wf, "im": im}
        for i in range(NCORES)
    ]
    res = run_bass_kernel_spmd(nc, in_maps, list(range(NCORES)))
    outs = [res.results[i]["out"] for i in range(NCORES)]
    full = np.concatenate(outs, axis=0).astype(np.float32)
    if _return_raw:
        return full, res
    return full
